# revision 39
# baseline (speedup 1.0000x reference)
"""Causal self-attention with RoPE on 8 trn2 NeuronCores.

Sharding: core c handles batch b = c//2 and head-half = c%2 (8 of 16 heads).
Each core computes its heads' attention output and a partial output
projection (row-slice of Wp); host sums the two partials per batch.

v2 layout/dtype notes (all matmuls 1 cyc/row):
  bf16 operands everywhere (PSUM accum fp32); fp32r only for the
  reciprocal-broadcast helper matmul.
  xT   [C=1024, T=2048] bf16, streamed twice (pairs 0,1 then 2,3)
  qT/kT per head-pair p: [128 = 2 heads x 64 dims (deinterleaved), T] bf16
  v    [T, 8*65] bf16, 65th col = ones (softmax denominator trick)
  S^T  [tk, tq] tiles, keys on partitions; diagonal tiles N-trimmed
  softmax denom -> ACT-engine reciprocal [1,512] -> K=1 ones-matmul
  broadcast [64,512] -> DVE normalize -> y kept in SBUF (s=1 heads moved
  to partitions 64:128 via small SBUF->SBUF DMA)
  phase B reads y from SBUF, interleaved into attention(3)
"""

import math
import sys

import numpy as np

for _p in ("/opt/trn_rl_repo",):
    if _p not in sys.path:
        sys.path.insert(0, _p)

B, T, C, H = 4, 2048, 1024, 16
D = C // H           # 64
HALF = D // 2        # 32
NCORES = 8
HPC = H // 2         # heads per core
NPAIR = HPC // 2     # head pairs per core
CK = C // 128        # 8 C-chunks
TCH = 512            # tq / T chunk width
NTCH = T // TCH      # 4
NTK = T // 128       # 16 tk tiles
N_WARM = 40          # warmup matmuls to lift the HAM clock gate


def build_nc():
    """Build the single-core SPMD Bass program (same NEFF on all 8 cores)."""
    import concourse.bass as bass
    import concourse.mybir as mybir
    import concourse.tile as tile
    from concourse.bass import ds, ts

    fp32 = mybir.dt.float32
    f32r = mybir.dt.float32r
    bf16 = mybir.dt.bfloat16
    Alu = mybir.AluOpType
    Act = mybir.ActivationFunctionType

    nc = bass.Bass("TRN2", target_bir_lowering=False, debug=False,
                   num_devices=NCORES)

    def din(name, shape, dt=None):
        return nc.dram_tensor(
            name, list(shape), dt or fp32, kind="ExternalInput").ap()

    xT_d = din("xT", (C, T), bf16)
    wq_d = din("wq", (C, HPC * D), bf16)
    wk_d = din("wk", (C, HPC * D), bf16)
    wv_d = din("wv", (C, HPC * D), bf16)
    wp_d = din("wp", (HPC * D, C), bf16)
    bq_d = din("bq_r", (128, NPAIR))
    bk_d = din("bk_r", (128, NPAIR))
    bvb_d = din("bv_b", (128, HPC * D))
    bpb_d = din("bp_b", (128, C))
    ropeA_d = din("ropeA", (128, T), bf16)
    ropeB_d = din("ropeB", (128, T), bf16)
    pswap_d = din("pswap", (128, 128), bf16)
    ones8_d = din("ones8", (128, HPC), bf16)
    tri_d = din("tri_neg", (128, 128), bf16)
    out_d = nc.dram_tensor("out", [T, C], fp32, kind="ExternalOutput").ap()

    with tile.TileContext(nc) as tc:
        with (
            tc.tile_pool(name="small", bufs=1) as small,
            tc.tile_pool(name="rpool", bufs=4) as rpool,
            tc.tile_pool(name="rdscr", bufs=4, space="DRAM") as rdscr,
            tc.tile_pool(name="ps", bufs=1, space="PSUM") as psp,
        ):
            # long-lived small constants
            bq_sb = small.tile([128, NPAIR], fp32)
            nc.sync.dma_start(bq_sb[:], bq_d[:, :])
            bk_sb = small.tile([128, NPAIR], fp32)
            nc.sync.dma_start(bk_sb[:], bk_d[:, :])
            bvb_sb = small.tile([128, HPC * D], fp32)
            nc.sync.dma_start(bvb_sb[:], bvb_d[:, :])
            bpb_sb = small.tile([128, C], fp32)
            nc.sync.dma_start(bpb_sb[:], bpb_d[:, :])
            pswap_sb = small.tile([128, 128], bf16)
            nc.sync.dma_start(pswap_sb[:], pswap_d[:, :])
            ones8_sb = small.tile([128, HPC], bf16)
            nc.sync.dma_start(ones8_sb[:], ones8_d[:, :])
            tri_sb = small.tile([128, 128], bf16)
            nc.sync.dma_start(tri_sb[:], tri_d[:, :])

            # warmup: keep the PE busy during the initial DMA wall so the
            # HAM clock gate opens before the first real matmul
            warm_sb = small.tile([128, TCH], bf16)
            nc.vector.memset(warm_sb[:], 0.0)
            for wi in range(N_WARM):
                wps = psp.tile([128, TCH], fp32, tag="mp", bufs=2,
                               name=f"warm{wi}")
                nc.tensor.matmul(wps[:], warm_sb[:, 0:128], warm_sb[:],
                                 start=True, stop=True)

            with (
                tc.tile_pool(name="biga", bufs=1) as biga,
                tc.tile_pool(name="xt", bufs=3) as xtp,
                tc.tile_pool(name="qk", bufs=6) as qkp,
                tc.tile_pool(name="raw", bufs=2) as rawp,
                tc.tile_pool(name="pt", bufs=6) as ptp,
                tc.tile_pool(name="ytm", bufs=2) as ytmp,
                tc.tile_pool(name="osb", bufs=3) as osbp,
            ):
                wq_sb = biga.tile([128, CK, HPC * D], bf16)
                wk_sb = biga.tile([128, CK, HPC * D], bf16)
                wv_sb = biga.tile([128, CK, HPC * D], bf16)
                wp_sb = biga.tile([128, NPAIR, C], bf16)
                ropeA_sb = biga.tile([128, T], bf16)
                ropeB_sb = biga.tile([128, T], bf16)
                v_sb = biga.tile([128, NTK, HPC * 65], bf16)
                v4 = v_sb[:].rearrange("p t (h e) -> p t h e", e=65)
                nc.vector.tensor_copy(
                    out=v4[:, :, :, 64:65],
                    in_=ones8_sb[:, None, :, None].to_broadcast(
                        (128, NTK, HPC, 1)))
                # y kept resident in SBUF: [2 heads x 64 dims, pair, T]
                yfull = biga.tile([128, NPAIR, T], bf16)

                qT = {}
                kT = {}

                xt_tiles = {}

                def xt_load(key, tcid):
                    """One DMA for a whole [C, 512] chunk of xT."""
                    xt_t = xtp.tile([128, CK, TCH], bf16, tag="xt",
                                    name=f"xt{key}")
                    nc.scalar.dma_start(
                        xt_t[:],
                        xT_d.rearrange("(k q) t -> q k t",
                                       q=128)[:, :, ts(tcid, TCH)])
                    xt_tiles[key] = xt_t

                def proj_chunk(pairs, tcid, with_v):
                    """One T-chunk of the projection pass: v (optional) +
                    q/k for the given pairs."""
                    if tcid == 0:
                        for p in pairs:
                            qT[p] = qkp.tile([128, T], bf16, tag="qk",
                                             name=f"qT{p}")
                            kT[p] = qkp.tile([128, T], bf16, tag="qk",
                                             name=f"kT{p}")
                    if True:
                        xt_t = xt_tiles[(with_v, tcid)]
                        xts = [xt_t[:, kc, :] for kc in range(CK)]
                        if with_v:
                            for tt in range(4):
                                tk = tcid * 4 + tt
                                pv = psp.tile([128, TCH], fp32, tag="mp",
                                              bufs=2)
                                for kc in range(CK):
                                    nc.tensor.matmul(
                                        pv[:],
                                        xts[kc][:, ts(tt, 128)],
                                        wv_sb[:, kc, :],
                                        start=(kc == 0), stop=(kc == CK - 1))
                                nc.vector.tensor_tensor(
                                    out=v4[:, tk, :, 0:64],
                                    in0=pv[:].rearrange("p (h e) -> p h e",
                                                        e=64),
                                    in1=bvb_sb[:].rearrange(
                                        "p (h e) -> p h e", e=64),
                                    op=Alu.add)
                        a_sl = ropeA_sb[:, ts(tcid, TCH)]
                        b_sl = ropeB_sb[:, ts(tcid, TCH)]
                        for p in pairs:
                            for (w_sb, b_sb, dst) in (
                                    (wq_sb, bq_sb, qT[p]),
                                    (wk_sb, bk_sb, kT[p])):
                                pq = psp.tile([128, TCH], fp32, tag="mp",
                                              bufs=2)
                                for kc in range(CK):
                                    nc.tensor.matmul(
                                        pq[:],
                                        w_sb[:, kc, ts(p, 128)],
                                        xts[kc][:],
                                        start=(kc == 0), stop=(kc == CK - 1))
                                raw = rawp.tile([128, TCH], bf16, tag="raw")
                                nc.vector.tensor_scalar_add(
                                    raw[:], pq[:], b_sb[:, p:p + 1])
                                psw = psp.tile([128, TCH], fp32, tag="mp",
                                               bufs=2)
                                nc.tensor.matmul(
                                    psw[:], pswap_sb[:], raw[:],
                                    start=True, stop=True)
                                tt_ = rawp.tile([128, TCH], bf16, tag="t")
                                nc.gpsimd.tensor_tensor(
                                    out=tt_[:], in0=raw[:], in1=a_sl[:],
                                    op=Alu.mult)
                                uu = rawp.tile([128, TCH], bf16, tag="u")
                                nc.vector.tensor_tensor(
                                    out=uu[:], in0=psw[:], in1=b_sl[:],
                                    op=Alu.mult)
                                nc.gpsimd.tensor_tensor(
                                    out=dst[:, ts(tcid, TCH)], in0=tt_[:],
                                    in1=uu[:], op=Alu.add)

                def phase_b_tile(t):
                    """out[t-tile] = y @ Wp + bp, straight from SBUF y."""
                    osb = osbp.tile([128, C], fp32, tag="osb")
                    for n in range(2):
                        po = psp.tile([128, TCH], fp32, tag="mp", bufs=2)
                        for p in range(NPAIR):
                            nc.tensor.matmul(
                                po[:], yfull[:, p, ts(t, 128)],
                                wp_sb[:, p, ts(n, TCH)],
                                start=(p == 0), stop=(p == NPAIR - 1))
                        nc.vector.tensor_tensor(
                            out=osb[:, ts(n, TCH)], in0=po[:],
                            in1=bpb_sb[:, ts(n, TCH)], op=Alu.add)
                    nc.sync.dma_start(out_d[ts(t, 128), :], osb[:])

                def attention_block(p, j):
                    """Attention for head pair p (heads 2p, 2p+1), query
                    chunk j."""
                    if True:
                        n_tk = 4 * j + 4
                        pvps = [psp.tile([65, TCH], fp32, tag="pv", bufs=2,
                                         name=f"pv{s_}") for s_ in range(2)]
                        for g in range(n_tk // 2):
                            sps = []
                            pts = []
                            los = []
                            for ti in (0, 1):
                                tk = 2 * g + ti
                                i = tk - 4 * j
                                los.append(128 * i if i > 0 else 0)
                            for s in range(2):
                                sps.append(psp.tile(
                                    [128, 2, TCH], fp32, tag="sp", bufs=2,
                                    name=f"sp{s}"))
                            # s-adjacent emission: the two heads' K=64
                            # matmuls occupy disjoint row groups and overlap
                            for ti in (0, 1):
                                tk = 2 * g + ti
                                lo = los[ti]
                                for s in range(2):
                                    row = ds(64 * s, 64)
                                    nc.tensor.matmul(
                                        sps[s][:, ti, lo:TCH],
                                        kT[p][row, ts(tk, 128)],
                                        qT[p][row, ds(TCH * j + lo,
                                                      TCH - lo)],
                                        start=True, stop=True)
                            for s in range(2):
                                pt = ptp.tile([128, 2, TCH], bf16,
                                              tag="pt")
                                pts.append(pt)
                                # one flat exp over both tiles; leading
                                # fully-masked columns skipped when both
                                # tiles sit past the diagonal
                                fl = los[0]
                                nc.scalar.activation(
                                    pt[:].rearrange(
                                        "p a b -> p (a b)")[:, fl:],
                                    sps[s][:].rearrange(
                                        "p a b -> p (a b)")[:, fl:],
                                    Act.Exp, scale=1.0 / math.sqrt(D))
                                for ti in (0, 1):
                                    tk = 2 * g + ti
                                    i = tk - 4 * j
                                    if i >= 0:
                                        cs = ds(128 * i, 128)
                                        nc.gpsimd.tensor_tensor(
                                            out=pt[:, ti, cs],
                                            in0=pt[:, ti, cs],
                                            in1=tri_sb[:], op=Alu.mult)
                            for s in range(2):
                                hs = 2 * p + s
                                for ti in (0, 1):
                                    tk = 2 * g + ti
                                    i = tk - 4 * j
                                    lo = 128 * i if i >= 0 else 0
                                    nc.tensor.matmul(
                                        pvps[s][:, lo:TCH],
                                        v_sb[:, tk, 65 * hs:65 * hs + 65],
                                        pts[s][:, ti, lo:TCH],
                                        start=(tk == 0),
                                        stop=(tk == n_tk - 1))
                        # Evacuate PSUM fast: copy unnormalized y + denom
                        # rows out right after PV so the next chunk's PV can
                        # reuse the banks; the whole reciprocal/broadcast
                        # chain then runs off the critical path and yfull is
                        # normalized in place (both heads in one multiply).
                        den2 = rpool.tile([65, 2 * TCH], fp32, tag="den")
                        for s in range(2):
                            nc.vector.tensor_copy(
                                out=den2[64:65, ts(s, TCH)],
                                in_=pvps[s][64:65, :])
                            if s == 0:
                                nc.vector.tensor_copy(
                                    out=yfull[0:64, p, ts(j, TCH)],
                                    in_=pvps[s][0:64, :])
                            else:
                                ytm = ytmp.tile([64, TCH], bf16, tag="ytm")
                                nc.vector.tensor_copy(
                                    out=ytm[:], in_=pvps[s][0:64, :])
                                nc.sync.dma_start(
                                    yfull[64:128, p, ts(j, TCH)], ytm[:])
                        d_dr = rdscr.tile([1, 2 * TCH], fp32, tag="dd")
                        nc.sync.dma_start(d_dr[:], den2[64:65, :])
                        rT = rpool.tile([128, 2, 4], fp32, tag="rT")
                        d_sc = bass.AP(
                            tensor=d_dr.tensor, offset=d_dr.offset,
                            ap=[[4, 128], [TCH, 2], [1, 4]])
                        nc.sync.dma_start(rT[:], d_sc)
                        rTi = rpool.tile([128, 2, 4], fp32, tag="rTi")
                        nc.vector.reciprocal(out=rTi[:], in_=rT[:])
                        r_dr = rdscr.tile([2, TCH], fp32, tag="rd")
                        r_ga = bass.AP(
                            tensor=r_dr.tensor, offset=r_dr.offset,
                            ap=[[4, 128], [TCH, 2], [1, 4]])
                        nc.sync.dma_start(r_ga, rTi[:])
                        rb_t = rpool.tile([128, TCH], fp32, tag="rb")
                        r_bc_ap = bass.AP(
                            tensor=r_dr.tensor, offset=r_dr.offset,
                            ap=[[TCH, 2], [0, 64], [1, TCH]])
                        nc.sync.dma_start(rb_t[:], r_bc_ap)
                        if p >= 2:
                            # pair 3's normalize gates phase B: split it per
                            # t-tile so phase B tiles start as soon as their
                            # slice is ready
                            for q4 in range(4):
                                cs4 = ds(TCH * j + 128 * q4, 128)
                                nc.vector.tensor_tensor(
                                    out=yfull[:, p, cs4],
                                    in0=yfull[:, p, cs4],
                                    in1=rb_t[:, ds(128 * q4, 128)],
                                    op=Alu.mult)
                        else:
                            nc.vector.tensor_tensor(
                                out=yfull[:, p, ts(j, TCH)],
                                in0=yfull[:, p, ts(j, TCH)], in1=rb_t[:],
                                op=Alu.mult)

                # Initial loads: x chunk 0, then kc-0 weight chunks (first
                # matmul needs only these), then the rest.
                w_rearr = [(wv_sb, wv_d.rearrange("(k q) f -> q k f", q=128)),
                           (wq_sb, wq_d.rearrange("(k q) f -> q k f", q=128)),
                           (wk_sb, wk_d.rearrange("(k q) f -> q k f", q=128))]
                xt_load((True, 0), 0)
                for w_sb, w_r in w_rearr:
                    nc.scalar.dma_start(w_sb[:, 0, :], w_r[:, 0, :])
                for w_sb, w_r in w_rearr:
                    nc.scalar.dma_start(w_sb[:, 1:CK, :], w_r[:, 1:CK, :])
                nc.scalar.dma_start(ropeA_sb[:], ropeA_d[:, :])
                nc.scalar.dma_start(ropeB_sb[:], ropeB_d[:, :])

                # staircase pipeline: each projection chunk immediately
                # unblocks the attention blocks for that query chunk, so the
                # PE never sits in a low-duty (ACT-paced) stretch and the
                # HAM clock gate stays open. The next chunk's bulk loads are
                # emitted BEFORE the attention blocks' normalizer DMAs: the
                # Sync queue executes triggers in order and a stalled
                # normalizer hop would otherwise block them.
                for tcid in range(NTCH):
                    proj_chunk((0, 1), tcid, with_v=True)
                    if tcid + 1 < NTCH:
                        xt_load((True, tcid + 1), tcid + 1)
                    else:
                        xt_load((False, 0), 0)
                        for pp in range(NPAIR):
                            nc.scalar.dma_start(
                                wp_sb[:, pp, :],
                                wp_d.rearrange("(p q) n -> q p n",
                                               q=128)[:, pp, :])
                    attention_block(0, tcid)
                    attention_block(1, tcid)
                for tcid in range(NTCH):
                    proj_chunk((2, 3), tcid, with_v=False)
                    if tcid + 1 < NTCH:
                        xt_load((False, tcid + 1), tcid + 1)
                    attention_block(2, tcid)
                    attention_block(3, tcid)
                    for t in range(4 * tcid, 4 * tcid + 4):
                        phase_b_tile(t)
    _split_drain_waits(nc, mybir)
    return nc


def _split_drain_waits(nc, mybir, max_w=1):
    """This walrus build allows at most one embedded sync wait per
    instruction (CTRL_NO for drains, S3_LW for matmuls, ...). Hoist all but
    the last wait of every instruction into standalone EventSemaphore
    instructions on the same engine, inserted immediately before it."""
    import bass_rust

    for f in nc.m.functions:
        for blk in f.blocks:
            insts = list(blk.instructions)
            out = []
            changed = False
            for ins in insts:
                si = ins.sync_info
                if si is not None and si.on_wait and len(si.on_wait) > max_w:
                    changed = True
                    waits = list(si.on_wait)
                    extra, keep = waits[:-max_w], waits[-max_w:]
                    for wi, w in enumerate(extra):
                        ev = mybir.InstEventSemaphore(
                            name=f"{ins.name}_w{wi}",
                            engine=ins.engine,
                            ins=[], outs=[],
                            debug=ins.debug,
                            sync_info=bass_rust.SyncInfo(
                                on_wait=[w], on_update=[]),
                        )
                        nc.register_instruction(ev, overwrite=True)
                        out.append(ev)
                    si.on_wait = keep
                    ins.sync_info = si
                out.append(ins)
            if changed:
                blk.instructions = out


def host_inputs(x, Wq, bq, Wk, bk, Wv, bv, Wp, bp):
    """Build the 8 per-core input maps."""
    import ml_dtypes
    bf = ml_dtypes.bfloat16
    half = D // 2
    perm = np.concatenate([np.arange(0, D, 2), np.arange(1, D, 2)])  # even|odd
    pos = np.arange(T, dtype=np.float32)[:, None]
    freqs = np.exp(np.arange(half, dtype=np.float32)
                   * np.float32(-math.log(10000.0) / (half - 1)))[None, :]
    args = pos * freqs                      # [T, 32]
    cos = np.cos(args).astype(np.float32).T   # [32, T]
    sin = np.sin(args).astype(np.float32).T
    ropeA = np.concatenate([cos, cos, cos, cos], 0).astype(bf)    # [128, T]
    ropeB = np.concatenate([-sin, sin, -sin, sin], 0).astype(bf)
    pswap = np.zeros((128, 128), np.float32)
    for blk in range(4):
        b0 = 32 * blk
        src = 32 * (blk ^ 1)
        for i in range(32):
            pswap[b0 + i, src + i] = 1.0
    pswap = pswap.astype(bf)
    r_idx = np.arange(128)[:, None]
    c_idx = np.arange(128)[None, :]
    tri = np.where(r_idx <= c_idx, 1.0, 0.0).astype(bf)

    in_maps = []
    for core in range(NCORES):
        b = core // 2
        h0 = (core % 2) * HPC
        cols = []
        for p in range(NPAIR):
            for hh in range(2):
                h = h0 + 2 * p + hh
                cols.append(h * D + perm)
        cols = np.concatenate(cols)           # deinterleaved q/k columns
        vcols = np.arange(h0 * D, (h0 + HPC) * D)
        bq_r = np.ascontiguousarray(
            bq[cols].reshape(NPAIR, 128).T)   # [128, 4]
        bk_r = np.ascontiguousarray(bk[cols].reshape(NPAIR, 128).T)
        bp_core = bp if core % 2 == 0 else np.zeros_like(bp)
        in_maps.append({
            "xT": np.ascontiguousarray(x[b].T).astype(bf),
            "wq": np.ascontiguousarray(Wq[:, cols]).astype(bf),
            "wk": np.ascontiguousarray(Wk[:, cols]).astype(bf),
            "wv": np.ascontiguousarray(Wv[:, vcols]).astype(bf),
            "wp": np.ascontiguousarray(Wp[vcols, :]).astype(bf),
            "bq_r": bq_r,
            "bk_r": bk_r,
            "bv_b": np.broadcast_to(bv[vcols], (128, HPC * D)).copy(),
            "bp_b": np.broadcast_to(bp_core, (128, C)).copy(),
            "ropeA": ropeA,
            "ropeB": ropeB,
            "pswap": pswap,
            "ones8": np.ones((128, HPC), bf),
            "tri_neg": tri,
        })
    return in_maps


_CACHE = {}
_PROFILE = False


def kernel(**inputs) -> np.ndarray:
    x = np.asarray(inputs["x"], np.float32)
    in_maps = host_inputs(
        x, *(np.asarray(inputs[k], np.float32) for k in
             ("Wq", "bq", "Wk", "bk", "Wv", "bv", "Wp", "bp")))
    from concourse.bass_utils import run_bass_kernel_spmd
    if "nc" not in _CACHE:
        _CACHE["nc"] = build_nc()
    bkr = run_bass_kernel_spmd(
        _CACHE["nc"], in_maps, core_ids=list(range(NCORES)),
        trace=_PROFILE)
    _CACHE["last"] = bkr
    res = bkr.results
    out = np.empty((B, T, C), np.float32)
    for b in range(B):
        out[b] = res[2 * b]["out"] + res[2 * b + 1]["out"]
    return out


# revision 41
# speedup vs baseline: 1.1734x; 1.1734x over previous
"""Causal self-attention with RoPE on 8 trn2 NeuronCores.

Sharding: core c handles batch b = c//2 and head-half = c%2 (8 of 16 heads).
Each core computes its heads' attention output and a partial output
projection (row-slice of Wp); host sums the two partials per batch.

v2 layout/dtype notes (all matmuls 1 cyc/row):
  bf16 operands everywhere (PSUM accum fp32); fp32r only for the
  reciprocal-broadcast helper matmul.
  xT   [C=1024, T=2048] bf16, streamed twice (pairs 0,1 then 2,3)
  qT/kT per head-pair p: [128 = 2 heads x 64 dims (deinterleaved), T] bf16
  v    [T, 8*65] bf16, 65th col = ones (softmax denominator trick)
  S^T  [tk, tq] tiles, keys on partitions; diagonal tiles N-trimmed
  softmax denom -> ACT-engine reciprocal [1,512] -> K=1 ones-matmul
  broadcast [64,512] -> DVE normalize -> y kept in SBUF (s=1 heads moved
  to partitions 64:128 via small SBUF->SBUF DMA)
  phase B reads y from SBUF, interleaved into attention(3)
"""

import math
import sys

import numpy as np

for _p in ("/opt/trn_rl_repo",):
    if _p not in sys.path:
        sys.path.insert(0, _p)

B, T, C, H = 4, 2048, 1024, 16
D = C // H           # 64
HALF = D // 2        # 32
NCORES = 8
HPC = H // 2         # heads per core
NPAIR = HPC // 2     # head pairs per core
CK = C // 128        # 8 C-chunks
TCH = 512            # tq / T chunk width
NTCH = T // TCH      # 4
NTK = T // 128       # 16 tk tiles
N_WARM = 40          # warmup matmuls to lift the HAM clock gate


def build_nc():
    """Build the single-core SPMD Bass program (same NEFF on all 8 cores)."""
    import concourse.bass as bass
    import concourse.mybir as mybir
    import concourse.tile as tile
    from concourse.bass import ds, ts

    fp32 = mybir.dt.float32
    f32r = mybir.dt.float32r
    bf16 = mybir.dt.bfloat16
    Alu = mybir.AluOpType
    Act = mybir.ActivationFunctionType

    nc = bass.Bass("TRN2", target_bir_lowering=False, debug=False,
                   num_devices=NCORES)

    def din(name, shape, dt=None):
        return nc.dram_tensor(
            name, list(shape), dt or fp32, kind="ExternalInput").ap()

    xT_d = din("xT", (C, T), bf16)
    wq_d = din("wq", (C, HPC * D), bf16)
    wk_d = din("wk", (C, HPC * D), bf16)
    wv_d = din("wv", (C, HPC * D), bf16)
    wp_d = din("wp", (HPC * D, C), bf16)
    bq_d = din("bq_r", (128, NPAIR))
    bk_d = din("bk_r", (128, NPAIR))
    bvb_d = din("bv_b", (128, HPC * D))
    bpb_d = din("bp_b", (128, C))
    ropeA_d = din("ropeA", (128, T), bf16)
    ropeB_d = din("ropeB", (128, T), bf16)
    pswap_d = din("pswap", (128, 128), bf16)
    ones8_d = din("ones8", (128, HPC), bf16)
    tri_d = din("tri_neg", (128, 128), bf16)
    out_d = nc.dram_tensor("out", [T, C], fp32, kind="ExternalOutput").ap()

    with tile.TileContext(nc) as tc:
        with (
            tc.tile_pool(name="small", bufs=1) as small,
            tc.tile_pool(name="rpool", bufs=4) as rpool,
            tc.tile_pool(name="rdscr", bufs=4, space="DRAM") as rdscr,
            tc.tile_pool(name="ps", bufs=1, space="PSUM") as psp,
        ):
            # long-lived small constants
            bq_sb = small.tile([128, NPAIR], fp32)
            nc.sync.dma_start(bq_sb[:], bq_d[:, :])
            bk_sb = small.tile([128, NPAIR], fp32)
            nc.sync.dma_start(bk_sb[:], bk_d[:, :])
            bvb_sb = small.tile([128, HPC * D], fp32)
            nc.sync.dma_start(bvb_sb[:], bvb_d[:, :])
            bpb_sb = small.tile([128, C], fp32)
            nc.sync.dma_start(bpb_sb[:], bpb_d[:, :])
            pswap_sb = small.tile([128, 128], bf16)
            nc.sync.dma_start(pswap_sb[:], pswap_d[:, :])
            ones8_sb = small.tile([128, HPC], bf16)
            nc.sync.dma_start(ones8_sb[:], ones8_d[:, :])
            tri_sb = small.tile([128, 128], bf16)
            nc.sync.dma_start(tri_sb[:], tri_d[:, :])

            # warmup: keep the PE busy during the initial DMA wall so the
            # HAM clock gate opens before the first real matmul
            warm_sb = small.tile([128, TCH], bf16)
            nc.vector.memset(warm_sb[:], 0.0)
            for wi in range(N_WARM):
                wps = psp.tile([128, TCH], fp32, tag="mp", bufs=2,
                               name=f"warm{wi}")
                nc.tensor.matmul(wps[:], warm_sb[:, 0:128], warm_sb[:],
                                 start=True, stop=True)

            with (
                tc.tile_pool(name="biga", bufs=1) as biga,
                tc.tile_pool(name="xt", bufs=8) as xtp,
                tc.tile_pool(name="ab", bufs=4) as abp,
                tc.tile_pool(name="qk", bufs=6) as qkp,
                tc.tile_pool(name="raw", bufs=2) as rawp,
                tc.tile_pool(name="pt", bufs=6) as ptp,
                tc.tile_pool(name="ytm", bufs=2) as ytmp,
                tc.tile_pool(name="osb", bufs=3) as osbp,
            ):
                wq_sb = biga.tile([128, CK, HPC * D], bf16)
                wk_sb = biga.tile([128, CK, HPC * D], bf16)
                wv_sb = biga.tile([128, CK, HPC * D], bf16)
                wp_sb = biga.tile([128, NPAIR, C], bf16)
                v_sb = biga.tile([128, NTK, HPC * 65], bf16)
                v4 = v_sb[:].rearrange("p t (h e) -> p t h e", e=65)
                nc.vector.tensor_copy(
                    out=v4[:, :, :, 64:65],
                    in_=ones8_sb[:, None, :, None].to_broadcast(
                        (128, NTK, HPC, 1)))
                # y kept resident in SBUF: [2 heads x 64 dims, pair, T]
                yfull = biga.tile([128, NPAIR, T], bf16)

                qT = {}
                kT = {}

                def proj_pass(pairs, with_v):
                    """One streaming pass over xT: v (optional) + q/k for
                    the given pairs. On the first pass the weight-chunk DMAs
                    are emitted next to their first use so the first matmul
                    starts a couple of microseconds in."""
                    for p in pairs:
                        qT[p] = qkp.tile([128, T], bf16, tag="qk",
                                         name=f"qT{p}")
                        kT[p] = qkp.tile([128, T], bf16, tag="qk",
                                         name=f"kT{p}")
                    for tcid in range(NTCH):
                        xts = []
                        for kc in range(CK):
                            xt = xtp.tile([128, TCH], bf16, tag="xt")
                            nc.sync.dma_start(
                                xt[:], xT_d[ts(kc, 128), ts(tcid, TCH)])
                            xts.append(xt)
                            if with_v and tcid == 0:
                                for w_sb, w_d in ((wv_sb, wv_d),
                                                  (wq_sb, wq_d),
                                                  (wk_sb, wk_d)):
                                    nc.sync.dma_start(
                                        w_sb[:, kc, :],
                                        w_d.rearrange(
                                            "(k q) f -> q k f",
                                            q=128)[:, kc, :])
                        if with_v:
                            for tt in range(4):
                                tk = tcid * 4 + tt
                                pv = psp.tile([128, TCH], fp32, tag="mp",
                                              bufs=2)
                                for kc in range(CK):
                                    nc.tensor.matmul(
                                        pv[:],
                                        xts[kc][:, ts(tt, 128)],
                                        wv_sb[:, kc, :],
                                        start=(kc == 0), stop=(kc == CK - 1))
                                nc.vector.tensor_tensor(
                                    out=v4[:, tk, :, 0:64],
                                    in0=pv[:].rearrange("p (h e) -> p h e",
                                                        e=64),
                                    in1=bvb_sb[:].rearrange(
                                        "p (h e) -> p h e", e=64),
                                    op=Alu.add)
                        a_sl = abp.tile([128, TCH], bf16, tag="a")
                        nc.sync.dma_start(a_sl[:], ropeA_d[:, ts(tcid, TCH)])
                        b_sl = abp.tile([128, TCH], bf16, tag="b")
                        nc.sync.dma_start(b_sl[:], ropeB_d[:, ts(tcid, TCH)])
                        for p in pairs:
                            for (w_sb, b_sb, dst) in (
                                    (wq_sb, bq_sb, qT[p]),
                                    (wk_sb, bk_sb, kT[p])):
                                pq = psp.tile([128, TCH], fp32, tag="mp",
                                              bufs=2)
                                for kc in range(CK):
                                    nc.tensor.matmul(
                                        pq[:],
                                        w_sb[:, kc, ts(p, 128)],
                                        xts[kc][:],
                                        start=(kc == 0), stop=(kc == CK - 1))
                                raw = rawp.tile([128, TCH], bf16, tag="raw")
                                nc.vector.tensor_scalar_add(
                                    raw[:], pq[:], b_sb[:, p:p + 1])
                                psw = psp.tile([128, TCH], fp32, tag="mp",
                                               bufs=2)
                                nc.tensor.matmul(
                                    psw[:], pswap_sb[:], raw[:],
                                    start=True, stop=True)
                                tt_ = rawp.tile([128, TCH], bf16, tag="t")
                                nc.gpsimd.tensor_tensor(
                                    out=tt_[:], in0=raw[:], in1=a_sl[:],
                                    op=Alu.mult)
                                uu = rawp.tile([128, TCH], bf16, tag="u")
                                nc.vector.tensor_tensor(
                                    out=uu[:], in0=psw[:], in1=b_sl[:],
                                    op=Alu.mult)
                                nc.gpsimd.tensor_tensor(
                                    out=dst[:, ts(tcid, TCH)], in0=tt_[:],
                                    in1=uu[:], op=Alu.add)

                def phase_b_tile(t):
                    """out[t-tile] = y @ Wp + bp, straight from SBUF y."""
                    osb = osbp.tile([128, C], fp32, tag="osb")
                    for n in range(2):
                        po = psp.tile([128, TCH], fp32, tag="mp", bufs=2)
                        for p in range(NPAIR):
                            nc.tensor.matmul(
                                po[:], yfull[:, p, ts(t, 128)],
                                wp_sb[:, p, ts(n, TCH)],
                                start=(p == 0), stop=(p == NPAIR - 1))
                        nc.vector.tensor_tensor(
                            out=osb[:, ts(n, TCH)], in0=po[:],
                            in1=bpb_sb[:, ts(n, TCH)], op=Alu.add)
                    nc.sync.dma_start(out_d[ts(t, 128), :], osb[:])

                def attention_block(p, j):
                    """Attention for head pair p (heads 2p, 2p+1), query
                    chunk j."""
                    if True:
                        n_tk = 4 * j + 4
                        pvps = [psp.tile([65, TCH], fp32, tag="pv", bufs=2,
                                         name=f"pv{s_}") for s_ in range(2)]
                        for g in range(n_tk // 2):
                            sps = []
                            pts = []
                            los = []
                            for ti in (0, 1):
                                tk = 2 * g + ti
                                i = tk - 4 * j
                                los.append(128 * i if i > 0 else 0)
                            for s in range(2):
                                sps.append(psp.tile(
                                    [128, 2, TCH], fp32, tag="sp", bufs=2,
                                    name=f"sp{s}"))
                            # s-adjacent emission: the two heads' K=64
                            # matmuls occupy disjoint row groups and overlap
                            for ti in (0, 1):
                                tk = 2 * g + ti
                                lo = los[ti]
                                for s in range(2):
                                    row = ds(64 * s, 64)
                                    nc.tensor.matmul(
                                        sps[s][:, ti, lo:TCH],
                                        kT[p][row, ts(tk, 128)],
                                        qT[p][row, ds(TCH * j + lo,
                                                      TCH - lo)],
                                        start=True, stop=True)
                            for s in range(2):
                                pt = ptp.tile([128, 2, TCH], bf16,
                                              tag="pt")
                                pts.append(pt)
                                # one flat exp over both tiles; leading
                                # fully-masked columns skipped when both
                                # tiles sit past the diagonal
                                fl = los[0]
                                nc.scalar.activation(
                                    pt[:].rearrange(
                                        "p a b -> p (a b)")[:, fl:],
                                    sps[s][:].rearrange(
                                        "p a b -> p (a b)")[:, fl:],
                                    Act.Exp, scale=1.0 / math.sqrt(D))
                                for ti in (0, 1):
                                    tk = 2 * g + ti
                                    i = tk - 4 * j
                                    if i >= 0:
                                        cs = ds(128 * i, 128)
                                        nc.gpsimd.tensor_tensor(
                                            out=pt[:, ti, cs],
                                            in0=pt[:, ti, cs],
                                            in1=tri_sb[:], op=Alu.mult)
                            for s in range(2):
                                hs = 2 * p + s
                                for ti in (0, 1):
                                    tk = 2 * g + ti
                                    i = tk - 4 * j
                                    lo = 128 * i if i >= 0 else 0
                                    nc.tensor.matmul(
                                        pvps[s][:, lo:TCH],
                                        v_sb[:, tk, 65 * hs:65 * hs + 65],
                                        pts[s][:, ti, lo:TCH],
                                        start=(tk == 0),
                                        stop=(tk == n_tk - 1))
                        # Evacuate PSUM fast: copy unnormalized y + denom
                        # rows out right after PV so the next chunk's PV can
                        # reuse the banks; the whole reciprocal/broadcast
                        # chain then runs off the critical path and yfull is
                        # normalized in place (both heads in one multiply).
                        den2 = rpool.tile([65, 2 * TCH], fp32, tag="den")
                        for s in range(2):
                            nc.vector.tensor_copy(
                                out=den2[64:65, ts(s, TCH)],
                                in_=pvps[s][64:65, :])
                            if s == 0:
                                nc.vector.tensor_copy(
                                    out=yfull[0:64, p, ts(j, TCH)],
                                    in_=pvps[s][0:64, :])
                            else:
                                ytm = ytmp.tile([64, TCH], bf16, tag="ytm")
                                nc.vector.tensor_copy(
                                    out=ytm[:], in_=pvps[s][0:64, :])
                                nc.sync.dma_start(
                                    yfull[64:128, p, ts(j, TCH)], ytm[:])
                        d_dr = rdscr.tile([1, 2 * TCH], fp32, tag="dd")
                        nc.sync.dma_start(d_dr[:], den2[64:65, :])
                        rT = rpool.tile([128, 2, 4], fp32, tag="rT")
                        d_sc = bass.AP(
                            tensor=d_dr.tensor, offset=d_dr.offset,
                            ap=[[4, 128], [TCH, 2], [1, 4]])
                        nc.sync.dma_start(rT[:], d_sc)
                        rTi = rpool.tile([128, 2, 4], fp32, tag="rTi")
                        nc.vector.reciprocal(out=rTi[:], in_=rT[:])
                        r_dr = rdscr.tile([2, TCH], fp32, tag="rd")
                        r_ga = bass.AP(
                            tensor=r_dr.tensor, offset=r_dr.offset,
                            ap=[[4, 128], [TCH, 2], [1, 4]])
                        nc.sync.dma_start(r_ga, rTi[:])
                        rb_t = rpool.tile([128, TCH], fp32, tag="rb")
                        r_bc_ap = bass.AP(
                            tensor=r_dr.tensor, offset=r_dr.offset,
                            ap=[[TCH, 2], [0, 64], [1, TCH]])
                        nc.sync.dma_start(rb_t[:], r_bc_ap)
                        if p >= 2:
                            # pair 3's normalize gates phase B: split it per
                            # t-tile so phase B tiles start as soon as their
                            # slice is ready
                            for q4 in range(4):
                                cs4 = ds(TCH * j + 128 * q4, 128)
                                nc.vector.tensor_tensor(
                                    out=yfull[:, p, cs4],
                                    in0=yfull[:, p, cs4],
                                    in1=rb_t[:, ds(128 * q4, 128)],
                                    op=Alu.mult)
                        else:
                            nc.vector.tensor_tensor(
                                out=yfull[:, p, ts(j, TCH)],
                                in0=yfull[:, p, ts(j, TCH)], in1=rb_t[:],
                                op=Alu.mult)

                proj_pass((0, 1), with_v=True)
                # pair 0 emitted before the second projection pass so the
                # scheduler can weave them (attention is ACT-paced, the
                # projection is PE-paced)
                for j in range(NTCH):
                    attention_block(0, j)
                for pp in range(NPAIR):
                    nc.sync.dma_start(
                        wp_sb[:, pp, :],
                        wp_d.rearrange("(p q) n -> q p n", q=128)[:, pp, :])
                proj_pass((2, 3), with_v=False)
                # pairs 1-3 interleaved chunk-by-chunk: three pairs' matmuls
                # share the span, keeping PE duty high enough that the HAM
                # clock gate stays open; phase B weaves in per chunk
                for j in range(NTCH):
                    for p in (1, 2, 3):
                        attention_block(p, j)
                    for t in range(4 * j, 4 * j + 4):
                        phase_b_tile(t)
    _split_drain_waits(nc, mybir)
    return nc


def _split_drain_waits(nc, mybir, max_w=1):
    """This walrus build allows at most one embedded sync wait per
    instruction (CTRL_NO for drains, S3_LW for matmuls, ...). Hoist all but
    the last wait of every instruction into standalone EventSemaphore
    instructions on the same engine, inserted immediately before it."""
    import bass_rust

    for f in nc.m.functions:
        for blk in f.blocks:
            insts = list(blk.instructions)
            out = []
            changed = False
            for ins in insts:
                si = ins.sync_info
                if si is not None and si.on_wait and len(si.on_wait) > max_w:
                    changed = True
                    waits = list(si.on_wait)
                    extra, keep = waits[:-max_w], waits[-max_w:]
                    for wi, w in enumerate(extra):
                        ev = mybir.InstEventSemaphore(
                            name=f"{ins.name}_w{wi}",
                            engine=ins.engine,
                            ins=[], outs=[],
                            debug=ins.debug,
                            sync_info=bass_rust.SyncInfo(
                                on_wait=[w], on_update=[]),
                        )
                        nc.register_instruction(ev, overwrite=True)
                        out.append(ev)
                    si.on_wait = keep
                    ins.sync_info = si
                out.append(ins)
            if changed:
                blk.instructions = out


def host_inputs(x, Wq, bq, Wk, bk, Wv, bv, Wp, bp):
    """Build the 8 per-core input maps."""
    import ml_dtypes
    bf = ml_dtypes.bfloat16
    half = D // 2
    perm = np.concatenate([np.arange(0, D, 2), np.arange(1, D, 2)])  # even|odd
    pos = np.arange(T, dtype=np.float32)[:, None]
    freqs = np.exp(np.arange(half, dtype=np.float32)
                   * np.float32(-math.log(10000.0) / (half - 1)))[None, :]
    args = pos * freqs                      # [T, 32]
    cos = np.cos(args).astype(np.float32).T   # [32, T]
    sin = np.sin(args).astype(np.float32).T
    ropeA = np.concatenate([cos, cos, cos, cos], 0).astype(bf)    # [128, T]
    ropeB = np.concatenate([-sin, sin, -sin, sin], 0).astype(bf)
    pswap = np.zeros((128, 128), np.float32)
    for blk in range(4):
        b0 = 32 * blk
        src = 32 * (blk ^ 1)
        for i in range(32):
            pswap[b0 + i, src + i] = 1.0
    pswap = pswap.astype(bf)
    r_idx = np.arange(128)[:, None]
    c_idx = np.arange(128)[None, :]
    tri = np.where(r_idx <= c_idx, 1.0, 0.0).astype(bf)

    in_maps = []
    for core in range(NCORES):
        b = core // 2
        h0 = (core % 2) * HPC
        cols = []
        for p in range(NPAIR):
            for hh in range(2):
                h = h0 + 2 * p + hh
                cols.append(h * D + perm)
        cols = np.concatenate(cols)           # deinterleaved q/k columns
        vcols = np.arange(h0 * D, (h0 + HPC) * D)
        bq_r = np.ascontiguousarray(
            bq[cols].reshape(NPAIR, 128).T)   # [128, 4]
        bk_r = np.ascontiguousarray(bk[cols].reshape(NPAIR, 128).T)
        bp_core = bp if core % 2 == 0 else np.zeros_like(bp)
        in_maps.append({
            "xT": np.ascontiguousarray(x[b].T).astype(bf),
            "wq": np.ascontiguousarray(Wq[:, cols]).astype(bf),
            "wk": np.ascontiguousarray(Wk[:, cols]).astype(bf),
            "wv": np.ascontiguousarray(Wv[:, vcols]).astype(bf),
            "wp": np.ascontiguousarray(Wp[vcols, :]).astype(bf),
            "bq_r": bq_r,
            "bk_r": bk_r,
            "bv_b": np.broadcast_to(bv[vcols], (128, HPC * D)).copy(),
            "bp_b": np.broadcast_to(bp_core, (128, C)).copy(),
            "ropeA": ropeA,
            "ropeB": ropeB,
            "pswap": pswap,
            "ones8": np.ones((128, HPC), bf),
            "tri_neg": tri,
        })
    return in_maps


_CACHE = {}
_PROFILE = False


def kernel(**inputs) -> np.ndarray:
    x = np.asarray(inputs["x"], np.float32)
    in_maps = host_inputs(
        x, *(np.asarray(inputs[k], np.float32) for k in
             ("Wq", "bq", "Wk", "bk", "Wv", "bv", "Wp", "bp")))
    from concourse.bass_utils import run_bass_kernel_spmd
    if "nc" not in _CACHE:
        _CACHE["nc"] = build_nc()
    bkr = run_bass_kernel_spmd(
        _CACHE["nc"], in_maps, core_ids=list(range(NCORES)),
        trace=_PROFILE)
    _CACHE["last"] = bkr
    res = bkr.results
    out = np.empty((B, T, C), np.float32)
    for b in range(B):
        out[b] = res[2 * b]["out"] + res[2 * b + 1]["out"]
    return out
